# revision 1
# baseline (speedup 1.0000x reference)
"""Causal self-attention (B=2, T=2048, C=1024, H=16, D=64) on 8 TRN2 NeuronCores.

Sharding: core c handles batch b = c//4 and 4 heads hg = c%4 (heads 4*hg..4*hg+3).
Each core computes:
  qk^T = (x[b] @ W_qk_slice + b_qk_slice)^T        [512, 2048]  (q/k of its 4 heads, [d, T] layout)
  v    = x[b] @ W_v_slice + b_v_slice              [2048, 256]  (natural [T, d] layout)
  per head: s^T = k^T.T-chunks @ q^T (causal), p^T = exp(s^T/8) masked,
            y^T_ext = v_ext.T @ p^T  (ones column gives softmax row sums l)
            y^T = y^T_unnorm * (1/l broadcast)
  partial = y_heads @ W_proj[head rows, :]         [2048, 1024]
Host: out[b] = sum of the 4 partials for b, + b_proj.
"""

import sys

if "/opt/trn_rl_repo" not in sys.path:
    sys.path.insert(0, "/opt/trn_rl_repo")

from contextlib import ExitStack

import numpy as np

import concourse.bacc as bacc
import concourse.mybir as mybir
import concourse.tile as tile
from concourse.masks import make_identity, make_upper_triangular

N_CORES = 8
T = 2048
C = 1024
HL = 4            # local heads per core
D = 64            # head dim
QK = 2 * HL * D   # 512 q+k channels per core
V = HL * D        # 256 v channels per core
P = 128
NT = T // P       # 16 token tiles
NCC = C // P      # 8 contraction chunks
SCALE = D ** -0.5
f32 = mybir.dt.float32
AF = mybir.ActivationFunctionType


def _slices_512(start, end):
    """[start, end) split on the 512 grid."""
    out = []
    n0 = start
    while n0 < end:
        n1 = min(end, (n0 // 512 + 1) * 512)
        out.append((n0, n1))
        n0 = n1
    return out


def build():
    nc = bacc.Bacc("TRN2", target_bir_lowering=False, debug=False,
                   num_devices=N_CORES)

    x_ap = nc.dram_tensor("x_b", [T, C], f32, kind="ExternalInput").ap()
    w_qk_ap = nc.dram_tensor("w_qk", [C, QK], f32, kind="ExternalInput").ap()
    b_qk_ap = nc.dram_tensor("b_qk", [QK], f32, kind="ExternalInput").ap()
    w_v_ap = nc.dram_tensor("w_v", [C, V], f32, kind="ExternalInput").ap()
    b_v_ap = nc.dram_tensor("b_v", [V], f32, kind="ExternalInput").ap()
    w_proj_ap = nc.dram_tensor("w_proj", [V, C], f32, kind="ExternalInput").ap()
    out_ap = nc.dram_tensor("out_partial", [T, C], f32, kind="ExternalOutput").ap()

    with tile.TileContext(nc) as tc, ExitStack() as ctx:
        const_pool = ctx.enter_context(tc.tile_pool(name="const", bufs=1))
        identity = const_pool.tile([P, P], f32, tag="identity", name="identity")
        make_identity(nc, identity[:])
        # keep element [j, i] iff j <= i  (upper triangular incl diag)
        mask01 = const_pool.tile([P, P], f32, tag="mask01", name="mask01")
        make_upper_triangular(nc, mask01[:], val=1.0, diag=True)
        ones_row = const_pool.tile([1, P], f32, tag="ones", name="ones")
        nc.vector.memset(ones_row[:], 1.0)
        bqk_t = const_pool.tile([P, QK // P], f32, tag="bqk", name="bqk")
        bqk_view = b_qk_ap.rearrange("(m p o) -> m p o", p=P, o=1)
        for m in range(QK // P):
            nc.sync.dma_start(bqk_t[:, m:m + 1], bqk_view[m])
        bv_row = const_pool.tile([1, V], f32, tag="bv", name="bv")
        nc.sync.dma_start(bv_row[:], b_v_ap.rearrange("(o v) -> o v", o=1))

        # persistent intermediates
        qk_pool = ctx.enter_context(tc.tile_pool(name="qkp", bufs=1))
        qk_sb = [qk_pool.tile([P, T], f32, tag=f"qk{m}", name=f"qk{m}")
                 for m in range(QK // P)]
        v_pool = ctx.enter_context(tc.tile_pool(name="vp", bufs=1))
        v_sb = [v_pool.tile([P, HL * (D + 1)], f32, tag=f"v{t}", name=f"v{t}")
                for t in range(NT)]
        yT_pool = ctx.enter_context(tc.tile_pool(name="yTp", bufs=1))
        yT_sb = [yT_pool.tile([P, T], f32, tag=f"yT{i}", name=f"yT{i}")
                 for i in range(V // P)]

        # ---------------- Phase A: x^T, qk^T, v ----------------
        with ExitStack() as actx:
            wqk_pool = actx.enter_context(tc.tile_pool(name="wqk", bufs=1))
            wqk = [wqk_pool.tile([P, QK], f32, tag=f"wqk{c}", name=f"wqk{c}")
                   for c in range(NCC)]
            wv_pool = actx.enter_context(tc.tile_pool(name="wv", bufs=1))
            wv = [wv_pool.tile([P, V], f32, tag=f"wv{c}", name=f"wv{c}")
                  for c in range(NCC)]
            xt_pool = actx.enter_context(tc.tile_pool(name="xt", bufs=1))
            xT = [xt_pool.tile([P, T], f32, tag=f"xt{c}", name=f"xt{c}")
                  for c in range(NCC)]
            xnat_pool = actx.enter_context(tc.tile_pool(name="xnat", bufs=3))
            xtp_pool = actx.enter_context(
                tc.tile_pool(name="xtp", bufs=2, space="PSUM"))
            qkps_pool = actx.enter_context(
                tc.tile_pool(name="qkps", bufs=4, space="PSUM"))
            vps_pool = actx.enter_context(
                tc.tile_pool(name="vps", bufs=2, space="PSUM"))

            wqk_view = w_qk_ap.rearrange("(c p) n -> c p n", p=P)
            wv_view = w_v_ap.rearrange("(c p) n -> c p n", p=P)
            for c in range(NCC):
                nc.sync.dma_start(wqk[c][:], wqk_view[c])
                nc.sync.dma_start(wv[c][:], wv_view[c])

            x_view = x_ap.rearrange("(t p) n -> t p n", p=P)
            for g in range(4):
                for tt in range(4 * g, 4 * g + 4):
                    xn = xnat_pool.tile([P, C], f32, tag="xnat", name="xn")
                    nc.sync.dma_start(xn[:], x_view[tt])
                    for c in range(NCC):
                        xp = xtp_pool.tile([P, P], f32, tag="xtp", name="xp")
                        nc.tensor.transpose(
                            xp[:], xn[:, c * P:(c + 1) * P], identity[:])
                        dst = xT[c][:, tt * P:(tt + 1) * P]
                        if c % 2 == 0:
                            nc.vector.tensor_copy(dst, xp[:])
                        else:
                            nc.scalar.copy(dst, xp[:])
                gs0, gs1 = g * 512, (g + 1) * 512
                for m in range(QK // P):
                    ps = qkps_pool.tile([P, 512], f32, tag="qkps", name="ps")
                    for c in range(NCC):
                        nc.tensor.matmul(
                            ps[:], lhsT=wqk[c][:, m * P:(m + 1) * P],
                            rhs=xT[c][:, gs0:gs1],
                            start=(c == 0), stop=(c == NCC - 1))
                    nc.scalar.activation(
                        qk_sb[m][:, gs0:gs1], ps[:], AF.Identity,
                        bias=bqk_t[:, m:m + 1], scale=1.0)
                for tt in range(4 * g, 4 * g + 4):
                    vp = vps_pool.tile([P, V], f32, tag="vps", name="vp")
                    for c in range(NCC):
                        nc.tensor.matmul(
                            vp[:], lhsT=xT[c][:, tt * P:(tt + 1) * P],
                            rhs=wv[c][:], start=(c == 0), stop=False)
                    # bias as rank-1 update: ones[T,1] @ b_v[1,V]
                    nc.tensor.matmul(
                        vp[:], lhsT=ones_row[0:1, 0:P], rhs=bv_row[:],
                        start=False, stop=True)
                    v3 = v_sb[tt][:].rearrange("p (h e) -> p h e", e=D + 1)
                    nc.vector.tensor_copy(
                        v3[:, :, 0:D],
                        vp[:].rearrange("p (h d) -> p h d", d=D))
                    nc.vector.memset(v3[:, :, D:D + 1], 1.0)

        # ---------------- Phase B: attention per head ----------------
        with ExitStack() as bctx:
            pt_pool = bctx.enter_context(tc.tile_pool(name="pt", bufs=3))
            rr_pool = bctx.enter_context(tc.tile_pool(name="rr", bufs=2))
            rbc_pool = bctx.enter_context(tc.tile_pool(name="rbc", bufs=2))
            sps_pool = bctx.enter_context(
                tc.tile_pool(name="sps", bufs=3, space="PSUM"))
            yext_pool = bctx.enter_context(
                tc.tile_pool(name="yext", bufs=1, space="PSUM"))

            for h in range(HL):
                po = (h % 2) * D
                qT = qk_sb[h // 2][po:po + D, :]
                kT = qk_sb[HL // 2 + h // 2][po:po + D, :]
                yext = yext_pool.tile([D + 1, T], f32, tag="yext", name="yext")
                for c in range(NT):
                    q0 = c * P
                    pT = pt_pool.tile([P, T], f32, tag="pt", name="pT")
                    for (n0, n1) in _slices_512(q0, T):
                        sp = sps_pool.tile([P, n1 - n0], f32, tag="sps",
                                           name="sp")
                        nc.tensor.matmul(
                            sp[:], lhsT=kT[:, q0:q0 + P], rhs=qT[:, n0:n1],
                            start=True, stop=True)
                        nc.scalar.activation(
                            pT[:, n0:n1], sp[:], AF.Exp, bias=0.0, scale=SCALE)
                    # causal mask inside the diagonal block
                    nc.vector.tensor_mul(
                        pT[:, q0:q0 + P], pT[:, q0:q0 + P], mask01[:])
                    for (n0, n1) in _slices_512(q0, T):
                        nc.tensor.matmul(
                            yext[:, n0:n1],
                            lhsT=v_sb[c][:, h * (D + 1):(h + 1) * (D + 1)],
                            rhs=pT[:, n0:n1],
                            start=(c == 0), stop=(c == NT - 1),
                            skip_group_check=True)
                # normalize rows by l (last partition row of yext) and
                # store into yT in [d, T] layout
                for g2 in range(4):
                    s0, s1 = g2 * 512, (g2 + 1) * 512
                    rr = rr_pool.tile([1, 512], f32, tag="rr", name="rr")
                    nc.vector.reciprocal(rr[:], yext[D:D + 1, s0:s1])
                    bp = sps_pool.tile([D, 512], f32, tag="sps", name="bp")
                    nc.tensor.matmul(bp[:], lhsT=ones_row[0:1, 0:D], rhs=rr[:],
                                     start=True, stop=True)
                    rb = rbc_pool.tile([D, 512], f32, tag="rbc", name="rb")
                    nc.scalar.copy(rb[:], bp[:])
                    nc.vector.tensor_mul(
                        yT_sb[h // 2][po:po + D, s0:s1],
                        yext[0:D, s0:s1], rb[:])

        # ---------------- Phase C: output projection ----------------
        with ExitStack() as cctx:
            wp_pool = cctx.enter_context(tc.tile_pool(name="wp", bufs=1))
            wp = [wp_pool.tile([P, C], f32, tag=f"wp{k}", name=f"wp{k}")
                  for k in range(V // P)]
            osb_pool = cctx.enter_context(tc.tile_pool(name="osb", bufs=3))
            pp_pool = cctx.enter_context(
                tc.tile_pool(name="pp", bufs=2, space="PSUM"))

            wp_view = w_proj_ap.rearrange("(k p) n -> k p n", p=P)
            for k in range(V // P):
                nc.sync.dma_start(wp[k][:], wp_view[k])
            out_view = out_ap.rearrange("(t p) n -> t p n", p=P)
            for tt in range(NT):
                pp = pp_pool.tile([P, C], f32, tag="pp", name="pp")
                for k in range(V // P):
                    for n2 in range(2):
                        nc.tensor.matmul(
                            pp[:, n2 * 512:(n2 + 1) * 512],
                            lhsT=yT_sb[k][:, tt * P:(tt + 1) * P],
                            rhs=wp[k][:, n2 * 512:(n2 + 1) * 512],
                            start=(k == 0), stop=(k == V // P - 1))
                ob = osb_pool.tile([P, C], f32, tag="osb", name="ob")
                nc.scalar.copy(ob[:, 0:512], pp[:, 0:512])
                nc.vector.tensor_copy(ob[:, 512:C], pp[:, 512:C])
                nc.sync.dma_start(out_view[tt], ob[:])

    nc.compile()
    return nc


_NC = None


def _get_nc():
    global _NC
    if _NC is None:
        _NC = build()
    return _NC


def make_in_maps(x, W_qkv, b_qkv, W_proj):
    """Per-core input dicts (host-side sharding)."""
    x = np.asarray(x, dtype=np.float32)
    W_qkv = np.asarray(W_qkv, dtype=np.float32)
    b_qkv = np.asarray(b_qkv, dtype=np.float32)
    W_proj = np.asarray(W_proj, dtype=np.float32)
    in_maps = []
    for core in range(N_CORES):
        b = core // 4
        hg = core % 4
        q0 = 256 * hg
        k0 = C + 256 * hg
        v0 = 2 * C + 256 * hg
        in_maps.append({
            "x_b": np.ascontiguousarray(x[b]),
            "w_qk": np.ascontiguousarray(
                np.concatenate([W_qkv[:, q0:q0 + 256],
                                W_qkv[:, k0:k0 + 256]], axis=1)),
            "b_qk": np.ascontiguousarray(
                np.concatenate([b_qkv[q0:q0 + 256], b_qkv[k0:k0 + 256]])),
            "w_v": np.ascontiguousarray(W_qkv[:, v0:v0 + 256]),
            "b_v": np.ascontiguousarray(b_qkv[v0:v0 + 256]),
            "w_proj": np.ascontiguousarray(W_proj[256 * hg:256 * hg + 256, :]),
        })
    return in_maps


def combine(results, b_proj):
    """Host-side unshard: sum the 4 per-core partials per batch, add bias."""
    b_proj = np.asarray(b_proj, dtype=np.float32)
    out = np.empty((2, T, C), dtype=np.float32)
    for b in range(2):
        acc = results[4 * b]["out_partial"].astype(np.float32)
        for c in range(4 * b + 1, 4 * b + 4):
            acc = acc + results[c]["out_partial"]
        out[b] = acc + b_proj
    return out


def kernel(x, W_qkv, b_qkv, W_proj, b_proj):
    from concourse.bass_utils import run_bass_kernel_spmd

    nc = _get_nc()
    in_maps = make_in_maps(x, W_qkv, b_qkv, W_proj)
    res = run_bass_kernel_spmd(nc, in_maps, list(range(N_CORES)))
    return combine(res.results, b_proj)
